# revision 1
# baseline (speedup 1.0000x reference)
"""Trainium2 Bass kernel for CrossMerge3D.

Input ys: [B=2, S=12, C=96, 32, 32, 32] f32. For each (b, c):
  out = (m0 + perm_j(m1) + perm_k(m2)) / 12
where, with the 12 scans split into 3 groups of 4, each group combines as
  m_g = s0 + s1 + flip(s2 + s3)   (flip over the flattened 32^3 volume)
and group 1's volume is stored as (j,k,i), group 2's as (k,i,j); perm_j /
perm_k bring them back to (i,j,k).

Sharding: 8 cores = batch (2) x channel quarters (4) -> 24 channels/core.
No cross-core communication.

Per-core layout: 4 channels x 32 leading-spatial -> 128 SBUF partitions,
1024-wide free dim. All loads are plain mergeable scan-pair DMAs (1 MiB,
fast HWDGE descriptor path; reversed/multi-dim source APs cost ~6.4us
per trigger on the issuing sequencer vs ~0.65us for these). The flip
splits into a free-dim reversal (folded into the pair-sum's operand APs)
and a partition-block reversal (a bit-exact fp32 matmul against a
block-exchange matrix on the otherwise idle TensorEngine). perm_j /
perm_k are DVE 32x32 block transposes plus free-dim permuted APs.
Loads are spread across both HWDGE rings (SP + ACT).
"""

import numpy as np

_B, _S, _C, _D = 2, 12, 96, 32
_NCORE = 8
_CL = _C // 4          # 24 channels per core
_G = _CL // 4          # 6 macro tiles of 4 channels (128 partitions)
_FREE = _D * _D        # 1024

_nc = None


def _build_program():
    from concourse import bacc, tile, mybir

    f32 = mybir.dt.float32
    nc = bacc.Bacc(
        "TRN2", target_bir_lowering=False, debug=False, num_devices=_NCORE
    )
    ys = nc.dram_tensor("ys", [_S, _CL, _D, _D, _D], f32, kind="ExternalInput")
    out = nc.dram_tensor("out", [_CL, _D, _D, _D], f32, kind="ExternalOutput")
    ysa = ys.ap()
    outa = out.ap()

    with tile.TileContext(nc) as tc:
        with (
            tc.tile_pool(name="const", bufs=1) as cst,
            tc.tile_pool(name="io", bufs=2) as iop,
            tc.tile_pool(name="tmp", bufs=2) as tmp,
            tc.tile_pool(name="ps", bufs=1, space="PSUM") as ps,
        ):
            # 32-block exchange stationary (anti-diagonal per block)
            jblk = cst.tile([128, 128], f32, tag="jblk", name="jblk")
            nc.gpsimd.memset(jblk[:], 1.0)
            for b in range(4):
                nc.gpsimd.affine_select(
                    out=jblk[32 * b:32 * b + 32, :],
                    in_=jblk[32 * b:32 * b + 32, :],
                    compare_op=mybir.AluOpType.is_equal, fill=0.0,
                    base=-(32 * b + 31), pattern=[[1, 128]],
                    channel_multiplier=1,
                )

            for g in range(_G):
                cs = slice(4 * g, 4 * (g + 1))

                def load_pair(s, tag, eng, bufs):
                    t = iop.tile([128, 2 * _FREE], f32, tag=tag, name=tag,
                                 bufs=bufs)
                    src = ysa[s:s + 2, cs].rearrange(
                        "s c i j k -> (c i) s (j k)"
                    )
                    dst = t[:].rearrange("p (s f) -> p s f", s=2)
                    eng.dma_start(out=dst, in_=src)
                    return t

                pa = load_pair(0, "pa", nc.sync, 2)
                pr = load_pair(2, "pr", nc.scalar, 2)
                qa = load_pair(4, "qa", nc.sync, 2)
                qr = load_pair(6, "qr", nc.scalar, 2)
                ra = load_pair(8, "ra", nc.sync, 2)
                rr = load_pair(10, "rr", nc.scalar, 2)

                def fwd_sum(t):
                    # in-place into the first half (elementwise aligned)
                    h0, h1 = t[:, 0:_FREE], t[:, _FREE:2 * _FREE]
                    nc.vector.tensor_add(h0, h0, h1)
                    return h0

                def rev_sum(t, tag):
                    # free-dim-reversed pair sum; partition reversal is done
                    # later by the jblk matmul
                    rs = tmp.tile([128, _FREE], f32, tag=tag, name=tag)
                    nc.vector.tensor_add(rs[:], t[:, 0:_FREE][:, ::-1],
                                         t[:, _FREE:2 * _FREE][:, ::-1])
                    return rs

                fA = fwd_sum(pa)
                rA = rev_sum(pr, "rA")
                fB = fwd_sum(qa)
                rB = rev_sum(qr, "rB")
                fC = fwd_sum(ra)
                rC = rev_sum(rr, "rC")

                def flip(rs, name):
                    # partition-block reversal on the TensorEngine
                    pf = ps.tile([128, _FREE], f32, tag="psF", name=name,
                                 bufs=4)
                    for n0 in (0, 512):
                        nc.tensor.matmul(pf[:, n0:n0 + 512], jblk[:],
                                         rs[:][:, n0:n0 + 512],
                                         start=True, stop=True)
                    return pf

                pfA = flip(rA, "pfA")
                pfB = flip(rB, "pfB")
                pfC = flip(rC, "pfC")

                # combines: grp = fwd + flipped_rev (PSUM operand)
                nc.vector.tensor_add(rA[:], fA, pfA[:])
                nc.vector.tensor_add(rB[:], fB, pfB[:])
                nc.vector.tensor_add(rC[:], fC, pfC[:])

                # group 1 ((j,k,i)): 32x32 block transpose, then add with
                # (k,j)->(j,k) free permute
                tb = tmp.tile([128, _FREE], f32, tag="tb", name="tb")
                nc.vector.transpose(tb[:], rB[:])
                acc3 = rA[:].rearrange("p (a b) -> p a b", a=_D)
                tbp = tb[:].rearrange("p (a b) -> p a b", a=_D).transpose(
                    [0, 2, 1]
                )
                nc.vector.tensor_add(acc3, acc3, tbp)

                # group 2 ((k,i,j)): (i,j)->(j,i) free permute (ScalarE),
                # then 32x32 block transpose
                cp = tmp.tile([128, _FREE], f32, tag="cp", name="cp")
                rcp = rC[:].rearrange("p (a b) -> p a b", a=_D).transpose(
                    [0, 2, 1]
                )
                nc.scalar.copy(cp[:].rearrange("p (a b) -> p a b", a=_D), rcp)
                tcb = tmp.tile([128, _FREE], f32, tag="tcb", name="tcb")
                nc.vector.transpose(tcb[:], cp[:])
                nc.vector.tensor_add(rA[:], rA[:], tcb[:])

                o = tmp.tile([128, _FREE], f32, tag="o", name="o")
                nc.scalar.mul(o[:], rA[:], 1.0 / 12.0)
                nc.sync.dma_start(
                    out=outa[cs].rearrange("c i j k -> (c i) (j k)"), in_=o[:]
                )

    nc.compile()
    return nc


def kernel(ys):
    global _nc
    ys = np.ascontiguousarray(ys, dtype=np.float32)
    assert ys.shape == (_B, _S, _C, _D, _D, _D), ys.shape

    if _nc is None:
        _nc = _build_program()

    from concourse.bass_utils import run_bass_kernel_spmd

    in_maps = []
    for r in range(_NCORE):
        b, q = divmod(r, 4)
        shard = np.ascontiguousarray(ys[b, :, q * _CL:(q + 1) * _CL])
        in_maps.append({"ys": shard})

    res = run_bass_kernel_spmd(_nc, in_maps, list(range(_NCORE)))

    out = np.empty((_B, _C, _D, _D, _D), np.float32)
    for r in range(_NCORE):
        b, q = divmod(r, 4)
        out[b, q * _CL:(q + 1) * _CL] = res.results[r]["out"]

    if res.exec_time_ns is not None:
        print(f"HW exec time: {res.exec_time_ns} ns")
    return out



# revision 3
# speedup vs baseline: 1.0022x; 1.0022x over previous
"""Trainium2 Bass kernel for CrossMerge3D.

Input ys: [B=2, S=12, C=96, 32, 32, 32] f32. For each (b, c):
  out = (m0 + perm_j(m1) + perm_k(m2)) / 12
where, with the 12 scans split into 3 groups of 4, each group combines as
  m_g = s0 + s1 + flip(s2 + s3)   (flip over the flattened 32^3 volume)
and group 1's volume is stored as (j,k,i), group 2's as (k,i,j); perm_j /
perm_k bring them back to (i,j,k).

Sharding: 8 cores = batch (2) x channel quarters (4) -> 24 channels/core.
No cross-core communication.

Per-core layout: 4 channels x 32 leading-spatial -> 128 SBUF partitions,
1024-wide free dim, 6 macro tiles. Per group the pair sums run fp32->bf16
(fwd on DVE, rev on GpSimd with the free-dim half of the flip folded into
reversed source APs), then the TensorEngine combines
  psum = J*rev_bf + I*fwd_bf
with cheap bf16 matmuls (1 cycle/row): J is the 32-block-exchange matrix
(the partition half of the flip), I the identity. The epilogue per tile
is 2 ACT scaled PSUM->SBUF bf16 copies, 2 DVE 32x32 block transposes and
2 fused DVE adds; output is stored bf16 (within the 2e-2 gate) and
widened to f32 on the host. Loads are 2 MiB 4-scan group DMAs on both
HWDGE rings (SP + ACT); stores go through the GpSimd ring.
"""

import numpy as np

_B, _S, _C, _D = 2, 12, 96, 32
_NCORE = 8
_CL = _C // 4          # 24 channels per core
_G = _CL // 4          # 6 macro tiles of 4 channels (128 partitions)
_FREE = _D * _D        # 1024

_nc = None


def _build_program():
    from concourse import bacc, tile, mybir

    f32 = mybir.dt.float32
    bf16 = mybir.dt.bfloat16
    nc = bacc.Bacc(
        "TRN2", target_bir_lowering=False, debug=False, num_devices=_NCORE
    )
    ys = nc.dram_tensor("ys", [_S, _CL, _D, _D, _D], f32, kind="ExternalInput")
    out = nc.dram_tensor("out", [_CL, _D, _D, _D], bf16, kind="ExternalOutput")
    ysa = ys.ap()
    outa = out.ap()

    with tile.TileContext(nc) as tc:
        with (
            tc.tile_pool(name="const", bufs=1) as cst,
            tc.tile_pool(name="io", bufs=2) as iop,
            tc.tile_pool(name="tmp", bufs=2) as tmp,
            tc.tile_pool(name="ps", bufs=1, space="PSUM") as ps,
        ):
            # stationaries: identity and 32-block exchange (anti-diagonal
            # per block), bf16 with exact 1.0 entries
            wJ = cst.tile([128, 128], bf16, tag="wJ", name="wJ")
            nc.gpsimd.memset(wJ[:], 1.0)
            for b in range(4):
                nc.gpsimd.affine_select(
                    out=wJ[32 * b:32 * b + 32, :],
                    in_=wJ[32 * b:32 * b + 32, :],
                    compare_op=mybir.AluOpType.is_equal, fill=0.0,
                    base=-(32 * b + 31), pattern=[[1, 128]],
                    channel_multiplier=1,
                )
            wI = cst.tile([128, 128], bf16, tag="wI", name="wI")
            nc.gpsimd.memset(wI[:], 1.0)
            nc.gpsimd.affine_select(
                out=wI[:], in_=wI[:],
                compare_op=mybir.AluOpType.is_equal, fill=0.0,
                base=0, pattern=[[-1, 128]], channel_multiplier=1,
            )

            for g in range(_G):
                cs = slice(4 * g, 4 * (g + 1))

                def load_group(s, tag, eng):
                    t = iop.tile([128, 4 * _FREE], f32, tag=tag, name=tag)
                    src = ysa[s:s + 4, cs].rearrange(
                        "s c i j k -> (c i) s (j k)"
                    )
                    dst = t[:].rearrange("p (s f) -> p s f", s=4)
                    eng.dma_start(out=dst, in_=src)
                    return t

                # group order j, k, i so the late adds only wait on psI
                Lj = load_group(4, "Lj", nc.sync)
                Lk = load_group(8, "Lk", nc.scalar)
                Li = load_group(0, "Li", nc.sync)

                def merge(L, name):
                    # fwd = s0+s1 (DVE), rev = free-reversed s2+s3 (GpSimd),
                    # both bf16; then psum = J*rev + I*fwd on the PE
                    fwd = tmp.tile([128, _FREE], bf16, tag="f" + name,
                                   name="f" + name)
                    nc.vector.tensor_add(fwd[:], L[:, 0:_FREE],
                                         L[:, _FREE:2 * _FREE])
                    rev = tmp.tile([128, _FREE], bf16, tag="r" + name,
                                   name="r" + name)
                    nc.gpsimd.tensor_add(rev[:],
                                         L[:, 2 * _FREE:3 * _FREE][:, ::-1],
                                         L[:, 3 * _FREE:4 * _FREE][:, ::-1])
                    p = ps.tile([128, _FREE], f32, tag="ps", name="p" + name,
                                bufs=4)
                    for n0 in (0, 512):
                        c = slice(n0, n0 + 512)
                        nc.tensor.matmul(p[:, c], wJ[:], rev[:, c],
                                         start=True, stop=False)
                        nc.tensor.matmul(p[:, c], wI[:], fwd[:, c],
                                         start=False, stop=True)
                    return p

                psJ = merge(Lj, "J")   # [c,j; (k,i)]
                psK = merge(Lk, "K")   # [c,k; (i,j)]
                psI = merge(Li, "I")   # [c,i; (j,k)]

                # group 1 ((j,k,i)): scaled bf16 copy, then 32x32 block
                # transpose -> [c,i; (k,j)]
                t1 = tmp.tile([128, _FREE], bf16, tag="t1", name="t1")
                nc.scalar.mul(t1[:], psJ[:], 1.0 / 12.0)
                tb = tmp.tile([128, _FREE], bf16, tag="tb", name="tb")
                nc.vector.transpose(tb[:], t1[:])

                # group 2 ((k,i,j)): scaled (i,j)->(j,i) permute copy, then
                # block transpose -> [c,i; (j,k)]
                t2 = tmp.tile([128, _FREE], bf16, tag="t2", name="t2")
                nc.scalar.mul(
                    t2[:].rearrange("p (a b) -> p a b", a=_D),
                    psK[:].rearrange("p (a b) -> p a b", a=_D).transpose(
                        [0, 2, 1]
                    ),
                    1.0 / 12.0,
                )
                tcb = tmp.tile([128, _FREE], bf16, tag="tcb", name="tcb")
                nc.vector.transpose(tcb[:], t2[:])

                # o1 = psI/12 + tcb ; o = o1 + tb[(k,j)->(j,k)]
                o1 = tmp.tile([128, _FREE], bf16, tag="o1", name="o1")
                nc.vector.scalar_tensor_tensor(
                    o1[:], psI[:], 1.0 / 12.0, tcb[:],
                    op0=mybir.AluOpType.mult, op1=mybir.AluOpType.add,
                )
                o = tmp.tile([128, _FREE], bf16, tag="o", name="o")
                tbp = tb[:].rearrange("p (a b) -> p a b", a=_D).transpose(
                    [0, 2, 1]
                )
                nc.vector.scalar_tensor_tensor(
                    o[:].rearrange("p (a b) -> p a b", a=_D), o1[:].rearrange(
                        "p (a b) -> p a b", a=_D), 1.0, tbp,
                    op0=mybir.AluOpType.mult, op1=mybir.AluOpType.add,
                )
                nc.gpsimd.dma_start(
                    out=outa[cs].rearrange("c i j k -> (c i) (j k)"), in_=o[:]
                )

    nc.compile()
    return nc


def kernel(ys):
    global _nc
    ys = np.ascontiguousarray(ys, dtype=np.float32)
    assert ys.shape == (_B, _S, _C, _D, _D, _D), ys.shape

    if _nc is None:
        _nc = _build_program()

    from concourse.bass_utils import run_bass_kernel_spmd

    in_maps = []
    for r in range(_NCORE):
        b, q = divmod(r, 4)
        shard = np.ascontiguousarray(ys[b, :, q * _CL:(q + 1) * _CL])
        in_maps.append({"ys": shard})

    res = run_bass_kernel_spmd(_nc, in_maps, list(range(_NCORE)))

    out = np.empty((_B, _C, _D, _D, _D), np.float32)
    for r in range(_NCORE):
        b, q = divmod(r, 4)
        out[b, q * _CL:(q + 1) * _CL] = np.asarray(
            res.results[r]["out"], dtype=np.float32
        )

    if res.exec_time_ns is not None:
        print(f"HW exec time: {res.exec_time_ns} ns")
    return out


# revision 4
# speedup vs baseline: 1.0479x; 1.0456x over previous
"""Trainium2 Bass kernel for CrossMerge3D.

Input ys: [B=2, S=12, C=96, 32, 32, 32] f32. For each (b, c):
  out = (m0 + perm_j(m1) + perm_k(m2)) / 12
where, with the 12 scans split into 3 groups of 4, each group combines as
  m_g = s0 + s1 + flip(s2 + s3)   (flip over the flattened 32^3 volume)
and group 1's volume is stored as (j,k,i), group 2's as (k,i,j); perm_j /
perm_k bring them back to (i,j,k).

Sharding: 8 cores = batch (2) x channel quarters (4) -> 24 channels/core.
No cross-core communication.

Per-core layout: 4 channels x 32 leading-spatial -> 128 SBUF partitions,
1024-wide free dim, 6 macro tiles. Per group the fwd pair sum runs on
GpSimd (packed) and the rev pair sum on DVE with the free-dim half of
the flip folded into reversed source APs; both emit bf16. The
TensorEngine then combines
  psum = J*rev_bf + I*fwd_bf
with bf16 matmuls (1 cycle/row): J is the 32-block-exchange matrix (the
partition half of the flip), I the identity. The epilogue per tile is 4
ACT scaled PSUM/SBUF->SBUF bf16 copies, 2 DVE 32x32 block transposes
and 2 fused DVE adds that hit the 4x packed-bf16 DVE mode; output is
stored bf16 (within the 2e-2 gate) and widened to f32 on the host.

All 36 pair loads ride one HWDGE ring (SP) in exact consumption order so
the per-queue FIFO delivers tiles just-in-time; stores use the GpSimd
ring so they never queue behind future loads.
"""

import numpy as np

_B, _S, _C, _D = 2, 12, 96, 32
_NCORE = 8
_CL = _C // 4          # 24 channels per core
_G = _CL // 4          # 6 macro tiles of 4 channels (128 partitions)
_FREE = _D * _D        # 1024

_nc = None


def _build_program():
    from concourse import bacc, tile, mybir

    f32 = mybir.dt.float32
    bf16 = mybir.dt.bfloat16
    nc = bacc.Bacc(
        "TRN2", target_bir_lowering=False, debug=False, num_devices=_NCORE
    )
    ys = nc.dram_tensor("ys", [_S, _CL, _D, _D, _D], f32, kind="ExternalInput")
    out = nc.dram_tensor("out", [_CL, _D, _D, _D], bf16, kind="ExternalOutput")
    ysa = ys.ap()
    outa = out.ap()

    with tile.TileContext(nc) as tc:
        with (
            tc.tile_pool(name="const", bufs=1) as cst,
            tc.tile_pool(name="io", bufs=2) as iop,
            tc.tile_pool(name="tmp", bufs=2) as tmp,
            tc.tile_pool(name="ps", bufs=1, space="PSUM") as ps,
        ):
            # stationaries: identity and 32-block exchange (anti-diagonal
            # per block), bf16 with exact 1.0 entries
            wJ = cst.tile([128, 128], bf16, tag="wJ", name="wJ")
            nc.gpsimd.memset(wJ[:], 1.0)
            for b in range(4):
                nc.gpsimd.affine_select(
                    out=wJ[32 * b:32 * b + 32, :],
                    in_=wJ[32 * b:32 * b + 32, :],
                    compare_op=mybir.AluOpType.is_equal, fill=0.0,
                    base=-(32 * b + 31), pattern=[[1, 128]],
                    channel_multiplier=1,
                )
            wI = cst.tile([128, 128], bf16, tag="wI", name="wI")
            nc.gpsimd.memset(wI[:], 1.0)
            nc.gpsimd.affine_select(
                out=wI[:], in_=wI[:],
                compare_op=mybir.AluOpType.is_equal, fill=0.0,
                base=0, pattern=[[-1, 128]], channel_multiplier=1,
            )

            for g in range(_G):
                cs = slice(4 * g, 4 * (g + 1))

                def load_pair(s, tag):
                    t = iop.tile([128, 2 * _FREE], f32, tag=tag, name=tag)
                    src = ysa[s:s + 2, cs].rearrange(
                        "s c i j k -> (c i) s (j k)"
                    )
                    dst = t[:].rearrange("p (s f) -> p s f", s=2)
                    nc.sync.dma_start(out=dst, in_=src)
                    return t

                # group order j, k, i so the late adds only wait on psI;
                # single ring, consumption order
                Jf = load_pair(4, "Jf")
                Jr = load_pair(6, "Jr")
                Kf = load_pair(8, "Kf")
                Kr = load_pair(10, "Kr")
                If = load_pair(0, "If")
                Ir = load_pair(2, "Ir")

                def merge(Lf, Lr, name):
                    # fwd = s0+s1 (GpSimd, packed), rev = free-reversed
                    # s2+s3 (DVE), both bf16; psum = J*rev + I*fwd (PE)
                    fwd = tmp.tile([128, _FREE], bf16, tag="f" + name,
                                   name="f" + name)
                    nc.gpsimd.tensor_add(fwd[:], Lf[:, 0:_FREE],
                                         Lf[:, _FREE:2 * _FREE])
                    rev = tmp.tile([128, _FREE], bf16, tag="r" + name,
                                   name="r" + name)
                    nc.vector.tensor_add(rev[:],
                                         Lr[:, 0:_FREE][:, ::-1],
                                         Lr[:, _FREE:2 * _FREE][:, ::-1])
                    p = ps.tile([128, _FREE], f32, tag="ps", name="p" + name,
                                bufs=4)
                    for n0 in (0, 512):
                        c = slice(n0, n0 + 512)
                        nc.tensor.matmul(p[:, c], wJ[:], rev[:, c],
                                         start=True, stop=False)
                        nc.tensor.matmul(p[:, c], wI[:], fwd[:, c],
                                         start=False, stop=True)
                    return p

                psJ = merge(Jf, Jr, "J")   # [c,j; (k,i)]
                psK = merge(Kf, Kr, "K")   # [c,k; (i,j)]

                # group 1 ((j,k,i)): scaled bf16 copy, block transpose to
                # [c,i; (k,j)], then ACT (k,j)->(j,k) permute copy
                t1 = tmp.tile([128, _FREE], bf16, tag="t1", name="t1")
                nc.scalar.mul(t1[:], psJ[:], 1.0 / 12.0)
                tb = tmp.tile([128, _FREE], bf16, tag="tb", name="tb")
                nc.vector.transpose(tb[:], t1[:])
                tbp = tmp.tile([128, _FREE], bf16, tag="tbp", name="tbp")
                nc.scalar.copy(
                    tbp[:].rearrange("p (a b) -> p a b", a=_D),
                    tb[:].rearrange("p (a b) -> p a b", a=_D).transpose(
                        [0, 2, 1]
                    ),
                )

                # group 2 ((k,i,j)): scaled (i,j)->(j,i) permute copy, then
                # block transpose -> [c,i; (j,k)]
                t2 = tmp.tile([128, _FREE], bf16, tag="t2", name="t2")
                nc.scalar.mul(
                    t2[:].rearrange("p (a b) -> p a b", a=_D),
                    psK[:].rearrange("p (a b) -> p a b", a=_D).transpose(
                        [0, 2, 1]
                    ),
                    1.0 / 12.0,
                )
                tcb = tmp.tile([128, _FREE], bf16, tag="tcb", name="tcb")
                nc.vector.transpose(tcb[:], t2[:])

                psI = merge(If, Ir, "I")   # [c,i; (j,k)]

                # si = psI/12 (ACT), then two 4x packed-bf16 DVE adds
                si = tmp.tile([128, _FREE], bf16, tag="si", name="si")
                nc.scalar.mul(si[:], psI[:], 1.0 / 12.0)
                o1 = tmp.tile([128, _FREE], bf16, tag="o1", name="o1")
                nc.vector.scalar_tensor_tensor(
                    o1[:], si[:], 1.0, tcb[:],
                    op0=mybir.AluOpType.mult, op1=mybir.AluOpType.add,
                )
                o = tmp.tile([128, _FREE], bf16, tag="o", name="o")
                nc.vector.scalar_tensor_tensor(
                    o[:], o1[:], 1.0, tbp[:],
                    op0=mybir.AluOpType.mult, op1=mybir.AluOpType.add,
                )
                nc.gpsimd.dma_start(
                    out=outa[cs].rearrange("c i j k -> (c i) (j k)"), in_=o[:]
                )

    nc.compile()
    return nc


def kernel(ys):
    global _nc
    ys = np.ascontiguousarray(ys, dtype=np.float32)
    assert ys.shape == (_B, _S, _C, _D, _D, _D), ys.shape

    if _nc is None:
        _nc = _build_program()

    from concourse.bass_utils import run_bass_kernel_spmd

    in_maps = []
    for r in range(_NCORE):
        b, q = divmod(r, 4)
        shard = np.ascontiguousarray(ys[b, :, q * _CL:(q + 1) * _CL])
        in_maps.append({"ys": shard})

    res = run_bass_kernel_spmd(_nc, in_maps, list(range(_NCORE)))

    out = np.empty((_B, _C, _D, _D, _D), np.float32)
    for r in range(_NCORE):
        b, q = divmod(r, 4)
        out[b, q * _CL:(q + 1) * _CL] = np.asarray(
            res.results[r]["out"], dtype=np.float32
        )

    if res.exec_time_ns is not None:
        print(f"HW exec time: {res.exec_time_ns} ns")
    return out


# revision 5
# speedup vs baseline: 1.0506x; 1.0026x over previous
"""Trainium2 Bass kernel for CrossMerge3D.

Input ys: [B=2, S=12, C=96, 32, 32, 32] f32. For each (b, c):
  out = (m0 + perm_j(m1) + perm_k(m2)) / 12
where, with the 12 scans split into 3 groups of 4, each group combines as
  m_g = s0 + s1 + flip(s2 + s3)   (flip over the flattened 32^3 volume)
and group 1's volume is stored as (j,k,i), group 2's as (k,i,j); perm_j /
perm_k bring them back to (i,j,k).

Sharding: 8 cores = batch (2) x channel quarters (4) -> 24 channels/core.
No cross-core communication.

Per-core layout: 4 channels x 32 leading-spatial -> 128 SBUF partitions,
1024-wide free dim, 6 macro tiles. The 4-scan group loads are GpSimd
SWDGE casting DMAs (fp32 DRAM -> bf16 SBUF, striped over all 16 DMA
engines) on one queue in exact consumption order: SWDGE triggers are
~1.1us deterministic on the otherwise idle Pool sequencer, avoiding the
HWDGE ring backpressure that serialized earlier versions. All-bf16
tiles let both pair sums per group run as 4x-mode packed DVE
scalar_tensor_tensor ops (~0.35us), with the free-dim half of the flip
folded into reversed (stride -1, still "packed") source APs. The
TensorEngine combines  psum = J*rev + I*fwd  with bf16 matmuls
(1 cycle/row): J is the 32-block-exchange matrix (the partition half of
the flip), I the identity. The epilogue per tile is 4 ACT scaled
PSUM/SBUF->SBUF bf16 copies, 2 DVE 32x32 block transposes and 2 fused
4x-mode DVE adds; output is stored bf16 (within the 2e-2 gate) on the
idle SP HWDGE ring and widened to f32 on the host.
"""

import numpy as np

_B, _S, _C, _D = 2, 12, 96, 32
_NCORE = 8
_CL = _C // 4          # 24 channels per core
_G = _CL // 4          # 6 macro tiles of 4 channels (128 partitions)
_FREE = _D * _D        # 1024

_nc = None


def _build_program():
    from concourse import bacc, tile, mybir

    f32 = mybir.dt.float32
    bf16 = mybir.dt.bfloat16
    nc = bacc.Bacc(
        "TRN2", target_bir_lowering=False, debug=False, num_devices=_NCORE
    )
    ys = nc.dram_tensor("ys", [_S, _CL, _D, _D, _D], f32, kind="ExternalInput")
    out = nc.dram_tensor("out", [_CL, _D, _D, _D], bf16, kind="ExternalOutput")
    ysa = ys.ap()
    outa = out.ap()

    with tile.TileContext(nc) as tc:
        with (
            tc.tile_pool(name="const", bufs=1) as cst,
            tc.tile_pool(name="io", bufs=2) as iop,
            tc.tile_pool(name="tmp", bufs=2) as tmp,
            tc.tile_pool(name="ps", bufs=1, space="PSUM") as ps,
        ):
            # stationaries: identity and 32-block exchange (anti-diagonal
            # per block), bf16 with exact 1.0 entries
            wJ = cst.tile([128, 128], bf16, tag="wJ", name="wJ")
            nc.gpsimd.memset(wJ[:], 1.0)
            for b in range(4):
                nc.gpsimd.affine_select(
                    out=wJ[32 * b:32 * b + 32, :],
                    in_=wJ[32 * b:32 * b + 32, :],
                    compare_op=mybir.AluOpType.is_equal, fill=0.0,
                    base=-(32 * b + 31), pattern=[[1, 128]],
                    channel_multiplier=1,
                )
            wI = cst.tile([128, 128], bf16, tag="wI", name="wI")
            nc.gpsimd.memset(wI[:], 1.0)
            nc.gpsimd.affine_select(
                out=wI[:], in_=wI[:],
                compare_op=mybir.AluOpType.is_equal, fill=0.0,
                base=0, pattern=[[-1, 128]], channel_multiplier=1,
            )

            for g in range(_G):
                cs = slice(4 * g, 4 * (g + 1))

                def load_group(s, tag):
                    # casting DMA: fp32 DRAM -> bf16 SBUF (SWDGE / Pool)
                    t = iop.tile([128, 4 * _FREE], bf16, tag=tag, name=tag)
                    src = ysa[s:s + 4, cs].rearrange(
                        "s c i j k -> (c i) s (j k)"
                    )
                    dst = t[:].rearrange("p (s f) -> p s f", s=4)
                    nc.gpsimd.dma_start(out=dst, in_=src)
                    return t

                # group order j, k, i so the late adds only wait on psI
                Lj = load_group(4, "Lj")
                Lk = load_group(8, "Lk")
                Li = load_group(0, "Li")

                def merge(L, name):
                    # fwd = s0+s1, rev = free-reversed s2+s3, both 4x-mode
                    # packed bf16 DVE ops; then psum = J*rev + I*fwd (PE)
                    add = mybir.AluOpType.add
                    mult = mybir.AluOpType.mult
                    fwd = tmp.tile([128, _FREE], bf16, tag="f" + name,
                                   name="f" + name)
                    nc.vector.scalar_tensor_tensor(
                        fwd[:], L[:, 0:_FREE], 1.0, L[:, _FREE:2 * _FREE],
                        op0=mult, op1=add,
                    )
                    rev = tmp.tile([128, _FREE], bf16, tag="r" + name,
                                   name="r" + name)
                    nc.vector.scalar_tensor_tensor(
                        rev[:], L[:, 2 * _FREE:3 * _FREE][:, ::-1], 1.0,
                        L[:, 3 * _FREE:4 * _FREE][:, ::-1],
                        op0=mult, op1=add,
                    )
                    p = ps.tile([128, _FREE], f32, tag="ps", name="p" + name,
                                bufs=4)
                    for n0 in (0, 512):
                        c = slice(n0, n0 + 512)
                        nc.tensor.matmul(p[:, c], wJ[:], rev[:, c],
                                         start=True, stop=False)
                        nc.tensor.matmul(p[:, c], wI[:], fwd[:, c],
                                         start=False, stop=True)
                    return p

                psJ = merge(Lj, "J")   # [c,j; (k,i)]
                psK = merge(Lk, "K")   # [c,k; (i,j)]

                # group 1 ((j,k,i)): scaled bf16 copy, block transpose to
                # [c,i; (k,j)], then ACT (k,j)->(j,k) permute copy
                t1 = tmp.tile([128, _FREE], bf16, tag="t1", name="t1")
                nc.scalar.mul(t1[:], psJ[:], 1.0 / 12.0)
                tb = tmp.tile([128, _FREE], bf16, tag="tb", name="tb")
                nc.vector.transpose(tb[:], t1[:])
                tbp = tmp.tile([128, _FREE], bf16, tag="tbp", name="tbp")
                nc.scalar.copy(
                    tbp[:].rearrange("p (a b) -> p a b", a=_D),
                    tb[:].rearrange("p (a b) -> p a b", a=_D).transpose(
                        [0, 2, 1]
                    ),
                )

                # group 2 ((k,i,j)): scaled (i,j)->(j,i) permute copy, then
                # block transpose -> [c,i; (j,k)]
                t2 = tmp.tile([128, _FREE], bf16, tag="t2", name="t2")
                nc.scalar.mul(
                    t2[:].rearrange("p (a b) -> p a b", a=_D),
                    psK[:].rearrange("p (a b) -> p a b", a=_D).transpose(
                        [0, 2, 1]
                    ),
                    1.0 / 12.0,
                )
                tcb = tmp.tile([128, _FREE], bf16, tag="tcb", name="tcb")
                nc.vector.transpose(tcb[:], t2[:])

                psI = merge(Li, "I")   # [c,i; (j,k)]

                # si = psI/12 (ACT), then two 4x packed-bf16 DVE adds
                si = tmp.tile([128, _FREE], bf16, tag="si", name="si")
                nc.scalar.mul(si[:], psI[:], 1.0 / 12.0)
                o1 = tmp.tile([128, _FREE], bf16, tag="o1", name="o1")
                nc.vector.scalar_tensor_tensor(
                    o1[:], si[:], 1.0, tcb[:],
                    op0=mybir.AluOpType.mult, op1=mybir.AluOpType.add,
                )
                o = tmp.tile([128, _FREE], bf16, tag="o", name="o")
                nc.vector.scalar_tensor_tensor(
                    o[:], o1[:], 1.0, tbp[:],
                    op0=mybir.AluOpType.mult, op1=mybir.AluOpType.add,
                )
                nc.sync.dma_start(
                    out=outa[cs].rearrange("c i j k -> (c i) (j k)"), in_=o[:]
                )

    nc.compile()
    return nc


def kernel(ys):
    global _nc
    ys = np.ascontiguousarray(ys, dtype=np.float32)
    assert ys.shape == (_B, _S, _C, _D, _D, _D), ys.shape

    if _nc is None:
        _nc = _build_program()

    from concourse.bass_utils import run_bass_kernel_spmd

    in_maps = []
    for r in range(_NCORE):
        b, q = divmod(r, 4)
        shard = np.ascontiguousarray(ys[b, :, q * _CL:(q + 1) * _CL])
        in_maps.append({"ys": shard})

    res = run_bass_kernel_spmd(_nc, in_maps, list(range(_NCORE)))

    out = np.empty((_B, _C, _D, _D, _D), np.float32)
    for r in range(_NCORE):
        b, q = divmod(r, 4)
        out[b, q * _CL:(q + 1) * _CL] = np.asarray(
            res.results[r]["out"], dtype=np.float32
        )

    if res.exec_time_ns is not None:
        print(f"HW exec time: {res.exec_time_ns} ns")
    return out


# revision 7
# speedup vs baseline: 1.2450x; 1.1851x over previous
"""Trainium2 Bass kernel for CrossMerge3D.

Input ys: [B=2, S=12, C=96, 32, 32, 32] f32. For each (b, c):
  out = (m0 + perm_j(m1) + perm_k(m2)) / 12
where, with the 12 scans split into 3 groups of 4, each group combines as
  m_g = s0 + s1 + flip(s2 + s3)   (flip over the flattened 32^3 volume)
and group 1's volume is stored as (j,k,i), group 2's as (k,i,j); perm_j /
perm_k bring them back to (i,j,k).

Sharding: 8 cores = batch (2) x channel quarters (4) -> 24 channels/core.
No cross-core communication.

Per-core layout: 4 channels x 32 leading-spatial -> 128 SBUF partitions,
1024-wide free dim, 6 macro tiles. The 4-scan group loads are GpSimd
SWDGE casting DMAs (fp32 DRAM -> bf16 SBUF, striped over all 16 DMA
engines) on one queue in exact consumption order, avoiding HWDGE ring
backpressure. Both pair sums per group are packed-bf16 DVE tensor_adds
(2x perf mode); the free-dim half of the flip rides the PE moving-
operand APs (reversed reads have odd element offsets, which would break
the DVE 2x mode's 4B-alignment rule). The TensorEngine combines
  psum = J*rev + I*fwd
with bf16 matmuls (1 cycle/row) whose weights carry the 1/12 scale: J is
the 32-block-exchange matrix (the partition half of the flip), I the
identity. Epilogue per tile: DVE block-transpose of psJ (PSUM src), ACT
permute copies for the two permuted groups, and two DVE tensor_adds;
output is stored bf16 (within the 2e-2 gate) on the idle SP HWDGE ring
and widened to f32 on the host.
"""

import numpy as np

_B, _S, _C, _D = 2, 12, 96, 32
_NCORE = 8
_CL = _C // 4          # 24 channels per core
_G = _CL // 4          # 6 macro tiles of 4 channels (128 partitions)
_FREE = _D * _D        # 1024

_nc = None


def _build_program():
    from concourse import bacc, tile, mybir

    f32 = mybir.dt.float32
    bf16 = mybir.dt.bfloat16
    nc = bacc.Bacc(
        "TRN2", target_bir_lowering=False, debug=False, num_devices=_NCORE
    )
    ys = nc.dram_tensor("ys", [_S, _CL, _D, _D, _D], f32, kind="ExternalInput")
    out = nc.dram_tensor("out", [_CL, _D, _D, _D], bf16, kind="ExternalOutput")
    ysa = ys.ap()
    outa = out.ap()

    with tile.TileContext(nc) as tc:
        with (
            tc.tile_pool(name="const", bufs=1) as cst,
            tc.tile_pool(name="io", bufs=2) as iop,
            tc.tile_pool(name="tmp", bufs=2) as tmp,
            tc.tile_pool(name="ps", bufs=1, space="PSUM") as ps,
        ):
            # stationaries carrying the 1/12 output scale: identity and
            # 32-block exchange (anti-diagonal per block), bf16
            wJ = cst.tile([128, 128], bf16, tag="wJ", name="wJ")
            nc.gpsimd.memset(wJ[:], 1.0 / 12.0)
            for b in range(4):
                nc.gpsimd.affine_select(
                    out=wJ[32 * b:32 * b + 32, :],
                    in_=wJ[32 * b:32 * b + 32, :],
                    compare_op=mybir.AluOpType.is_equal, fill=0.0,
                    base=-(32 * b + 31), pattern=[[1, 128]],
                    channel_multiplier=1,
                )
            wI = cst.tile([128, 128], bf16, tag="wI", name="wI")
            nc.gpsimd.memset(wI[:], 1.0 / 12.0)
            nc.gpsimd.affine_select(
                out=wI[:], in_=wI[:],
                compare_op=mybir.AluOpType.is_equal, fill=0.0,
                base=0, pattern=[[-1, 128]], channel_multiplier=1,
            )

            for g in range(_G):
                cs = slice(4 * g, 4 * (g + 1))

                def load_group(s, tag):
                    # casting DMA: fp32 DRAM -> bf16 SBUF (SWDGE / Pool)
                    t = iop.tile([128, 4 * _FREE], bf16, tag=tag, name=tag)
                    src = ysa[s:s + 4, cs].rearrange(
                        "s c i j k -> (c i) s (j k)"
                    )
                    dst = t[:].rearrange("p (s f) -> p s f", s=4)
                    nc.gpsimd.dma_start(out=dst, in_=src)
                    return t

                # group order j, k, i so the late adds only wait on psI
                Lj = load_group(4, "Lj")
                Lk = load_group(8, "Lk")
                Li = load_group(0, "Li")

                def merge(L, name):
                    # fwd = s0+s1, rev = s2+s3: packed bf16 2x-mode DVE
                    # adds; psum = J*flip(rev) + I*fwd on the PE, with the
                    # free-dim reversal in the moving-operand AP
                    fwd = tmp.tile([128, _FREE], bf16, tag="f" + name,
                                   name="f" + name)
                    nc.vector.tensor_add(fwd[:], L[:, 0:_FREE],
                                         L[:, _FREE:2 * _FREE])
                    rev = tmp.tile([128, _FREE], bf16, tag="r" + name,
                                   name="r" + name)
                    nc.vector.tensor_add(rev[:], L[:, 2 * _FREE:3 * _FREE],
                                         L[:, 3 * _FREE:4 * _FREE])
                    revr = rev[:][:, ::-1]
                    p = ps.tile([128, _FREE], f32, tag="ps", name="p" + name,
                                bufs=4)
                    for n0 in (0, 512):
                        c = slice(n0, n0 + 512)
                        nc.tensor.matmul(p[:, c], wJ[:], revr[:, c],
                                         start=True, stop=False)
                        nc.tensor.matmul(p[:, c], wI[:], fwd[:, c],
                                         start=False, stop=True)
                    return p

                psJ = merge(Lj, "J")   # [c,j; (k,i)]
                psK = merge(Lk, "K")   # [c,k; (i,j)]

                # group 1 ((j,k,i)): ACT PSUM->SBUF bf16 copy, block
                # transpose to [c,i; (k,j)], then ACT (k,j)->(j,k) permute
                t1 = tmp.tile([128, _FREE], bf16, tag="t1", name="t1")
                nc.scalar.copy(t1[:], psJ[:])
                tb = tmp.tile([128, _FREE], bf16, tag="tb", name="tb")
                nc.vector.transpose(tb[:], t1[:])
                tbp = tmp.tile([128, _FREE], bf16, tag="tbp", name="tbp")
                nc.scalar.copy(
                    tbp[:].rearrange("p (a b) -> p a b", a=_D),
                    tb[:].rearrange("p (a b) -> p a b", a=_D).transpose(
                        [0, 2, 1]
                    ),
                )

                # group 2 ((k,i,j)): ACT (i,j)->(j,i) permute copy from
                # PSUM, then block transpose -> [c,i; (j,k)]
                t2 = tmp.tile([128, _FREE], bf16, tag="t2", name="t2")
                nc.scalar.copy(
                    t2[:].rearrange("p (a b) -> p a b", a=_D),
                    psK[:].rearrange("p (a b) -> p a b", a=_D).transpose(
                        [0, 2, 1]
                    ),
                )
                tcb = tmp.tile([128, _FREE], bf16, tag="tcb", name="tcb")
                nc.vector.transpose(tcb[:], t2[:])

                psI = merge(Li, "I")   # [c,i; (j,k)]

                # o1 = psI + tcb (PSUM src, 1x); o = o1 + tbp (2x)
                o1 = tmp.tile([128, _FREE], bf16, tag="o1", name="o1")
                nc.vector.tensor_add(o1[:], psI[:], tcb[:])
                o = tmp.tile([128, _FREE], bf16, tag="o", name="o")
                nc.vector.tensor_add(o[:], o1[:], tbp[:])
                nc.sync.dma_start(
                    out=outa[cs].rearrange("c i j k -> (c i) (j k)"), in_=o[:]
                )

    nc.compile()
    return nc


def kernel(ys):
    global _nc
    ys = np.ascontiguousarray(ys, dtype=np.float32)
    assert ys.shape == (_B, _S, _C, _D, _D, _D), ys.shape

    if _nc is None:
        _nc = _build_program()

    from concourse.bass_utils import run_bass_kernel_spmd

    in_maps = []
    for r in range(_NCORE):
        b, q = divmod(r, 4)
        shard = np.ascontiguousarray(ys[b, :, q * _CL:(q + 1) * _CL])
        in_maps.append({"ys": shard})

    res = run_bass_kernel_spmd(_nc, in_maps, list(range(_NCORE)))

    out = np.empty((_B, _C, _D, _D, _D), np.float32)
    for r in range(_NCORE):
        b, q = divmod(r, 4)
        out[b, q * _CL:(q + 1) * _CL] = np.asarray(
            res.results[r]["out"], dtype=np.float32
        )

    if res.exec_time_ns is not None:
        print(f"HW exec time: {res.exec_time_ns} ns")
    return out


# revision 9
# speedup vs baseline: 1.2528x; 1.0062x over previous
"""Trainium2 Bass kernel for CrossMerge3D.

Input ys: [B=2, S=12, C=96, 32, 32, 32] f32. For each (b, c):
  out = (m0 + perm_j(m1) + perm_k(m2)) / 12
where, with the 12 scans split into 3 groups of 4, each group combines as
  m_g = s0 + s1 + flip(s2 + s3)   (flip over the flattened 32^3 volume)
and group 1's volume is stored as (j,k,i), group 2's as (k,i,j); perm_j /
perm_k bring them back to (i,j,k).

Sharding: 8 cores = batch (2) x channel quarters (4) -> 24 channels/core.
No cross-core communication.

Per-core layout: 4 channels x 32 leading-spatial -> 128 SBUF partitions,
1024-wide free dim, 6 macro tiles. The 4-scan group loads are GpSimd
SWDGE casting DMAs (fp32 DRAM -> bf16 SBUF, striped over all 16 DMA
engines) on one queue in exact consumption order, avoiding HWDGE ring
backpressure. Both pair sums per group are packed-bf16 DVE tensor_adds
(2x perf mode); the free-dim half of the flip rides the PE moving-
operand APs (reversed reads have odd element offsets, which would break
the DVE 2x mode's 4B-alignment rule). The TensorEngine combines
  psum = J*rev + I*fwd
with bf16 matmuls (1 cycle/row) whose weights carry the 1/12 scale: J is
the 32-block-exchange matrix (the partition half of the flip), I the
identity. Epilogue per tile: DVE block-transpose of psJ (PSUM src), ACT
permute copies for the two permuted groups, and two DVE tensor_adds;
output is stored bf16 (within the 2e-2 gate) on the idle SP HWDGE ring
and widened to f32 on the host.
"""

import numpy as np

_B, _S, _C, _D = 2, 12, 96, 32
_NCORE = 8
_CL = _C // 4          # 24 channels per core
_G = _CL // 4          # 6 macro tiles of 4 channels (128 partitions)
_FREE = _D * _D        # 1024

_nc = None


def _build_program():
    from concourse import bacc, tile, mybir

    f32 = mybir.dt.float32
    bf16 = mybir.dt.bfloat16
    nc = bacc.Bacc(
        "TRN2", target_bir_lowering=False, debug=False, num_devices=_NCORE
    )
    ys = nc.dram_tensor("ys", [_S, _CL, _D, _D, _D], f32, kind="ExternalInput")
    out = nc.dram_tensor("out", [_CL, _D, _D, _D], bf16, kind="ExternalOutput")
    ysa = ys.ap()
    outa = out.ap()

    with tile.TileContext(nc) as tc:
        with (
            tc.tile_pool(name="const", bufs=1) as cst,
            tc.tile_pool(name="io", bufs=3) as iop,
            tc.tile_pool(name="tmp", bufs=2) as tmp,
            tc.tile_pool(name="ps", bufs=1, space="PSUM") as ps,
        ):
            def load_group(g, s, tag):
                # casting DMA: fp32 DRAM -> bf16 SBUF (SWDGE / Pool)
                cs = slice(4 * g, 4 * (g + 1))
                t = iop.tile([128, 4 * _FREE], bf16, tag=tag, name=tag)
                src = ysa[s:s + 4, cs].rearrange(
                    "s c i j k -> (c i) s (j k)"
                )
                dst = t[:].rearrange("p (s f) -> p s f", s=4)
                nc.gpsimd.dma_start(out=dst, in_=src)
                return t

            # first tile's loads go out before the constants build so the
            # DMA stream starts immediately
            Lj0 = load_group(0, 4, "Lj")
            Lk0 = load_group(0, 8, "Lk")
            Li0 = load_group(0, 0, "Li")

            # stationaries carrying the 1/12 output scale: identity and
            # 32-block exchange (anti-diagonal per block), bf16
            wJ = cst.tile([128, 128], bf16, tag="wJ", name="wJ")
            nc.gpsimd.memset(wJ[:], 1.0 / 12.0)
            for b in range(4):
                nc.gpsimd.affine_select(
                    out=wJ[32 * b:32 * b + 32, :],
                    in_=wJ[32 * b:32 * b + 32, :],
                    compare_op=mybir.AluOpType.is_equal, fill=0.0,
                    base=-(32 * b + 31), pattern=[[1, 128]],
                    channel_multiplier=1,
                )
            wI = cst.tile([128, 128], bf16, tag="wI", name="wI")
            nc.gpsimd.memset(wI[:], 1.0 / 12.0)
            nc.gpsimd.affine_select(
                out=wI[:], in_=wI[:],
                compare_op=mybir.AluOpType.is_equal, fill=0.0,
                base=0, pattern=[[-1, 128]], channel_multiplier=1,
            )

            for g in range(_G):
                cs = slice(4 * g, 4 * (g + 1))

                # group order j, k, i so the late adds only wait on psI
                if g == 0:
                    Lj, Lk, Li = Lj0, Lk0, Li0
                else:
                    Lj = load_group(g, 4, "Lj")
                    Lk = load_group(g, 8, "Lk")
                    Li = load_group(g, 0, "Li")

                def merge(L, name):
                    # fwd = s0+s1, rev = s2+s3: packed bf16 2x-mode DVE
                    # adds; psum = J*flip(rev) + I*fwd on the PE, with the
                    # free-dim reversal in the moving-operand AP
                    fwd = tmp.tile([128, _FREE], bf16, tag="f" + name,
                                   name="f" + name)
                    nc.vector.tensor_add(fwd[:], L[:, 0:_FREE],
                                         L[:, _FREE:2 * _FREE])
                    rev = tmp.tile([128, _FREE], bf16, tag="r" + name,
                                   name="r" + name)
                    nc.vector.tensor_add(rev[:], L[:, 2 * _FREE:3 * _FREE],
                                         L[:, 3 * _FREE:4 * _FREE])
                    revr = rev[:][:, ::-1]
                    p = ps.tile([128, _FREE], f32, tag="ps", name="p" + name,
                                bufs=4)
                    for n0 in (0, 512):
                        c = slice(n0, n0 + 512)
                        nc.tensor.matmul(p[:, c], wJ[:], revr[:, c],
                                         start=True, stop=False)
                        nc.tensor.matmul(p[:, c], wI[:], fwd[:, c],
                                         start=False, stop=True)
                    return p

                psJ = merge(Lj, "J")   # [c,j; (k,i)]
                psK = merge(Lk, "K")   # [c,k; (i,j)]

                # group 1 ((j,k,i)): ACT PSUM->SBUF bf16 copy, block
                # transpose to [c,i; (k,j)], then ACT (k,j)->(j,k) permute
                t1 = tmp.tile([128, _FREE], bf16, tag="t1", name="t1")
                nc.scalar.copy(t1[:], psJ[:])
                tb = tmp.tile([128, _FREE], bf16, tag="tb", name="tb")
                nc.vector.transpose(tb[:], t1[:])
                tbp = tmp.tile([128, _FREE], bf16, tag="tbp", name="tbp")
                nc.scalar.copy(
                    tbp[:].rearrange("p (a b) -> p a b", a=_D),
                    tb[:].rearrange("p (a b) -> p a b", a=_D).transpose(
                        [0, 2, 1]
                    ),
                )

                # group 2 ((k,i,j)): ACT (i,j)->(j,i) permute copy from
                # PSUM, then block transpose -> [c,i; (j,k)]
                t2 = tmp.tile([128, _FREE], bf16, tag="t2", name="t2")
                nc.scalar.copy(
                    t2[:].rearrange("p (a b) -> p a b", a=_D),
                    psK[:].rearrange("p (a b) -> p a b", a=_D).transpose(
                        [0, 2, 1]
                    ),
                )
                tcb = tmp.tile([128, _FREE], bf16, tag="tcb", name="tcb")
                nc.vector.transpose(tcb[:], t2[:])

                psI = merge(Li, "I")   # [c,i; (j,k)]

                # tc = tcb + tbp (2x, runs under the I-group matmuls, off
                # the critical chain); o = psI + tc (PSUM src, 1x)
                tc2 = tmp.tile([128, _FREE], bf16, tag="tc2", name="tc2")
                nc.vector.tensor_add(tc2[:], tcb[:], tbp[:])
                o = tmp.tile([128, _FREE], bf16, tag="o", name="o")
                nc.vector.tensor_add(o[:], psI[:], tc2[:])
                nc.sync.dma_start(
                    out=outa[cs].rearrange("c i j k -> (c i) (j k)"), in_=o[:]
                )

    nc.compile()
    return nc


def kernel(ys):
    global _nc
    ys = np.ascontiguousarray(ys, dtype=np.float32)
    assert ys.shape == (_B, _S, _C, _D, _D, _D), ys.shape

    if _nc is None:
        _nc = _build_program()

    from concourse.bass_utils import run_bass_kernel_spmd

    in_maps = []
    for r in range(_NCORE):
        b, q = divmod(r, 4)
        shard = np.ascontiguousarray(ys[b, :, q * _CL:(q + 1) * _CL])
        in_maps.append({"ys": shard})

    res = run_bass_kernel_spmd(_nc, in_maps, list(range(_NCORE)))

    out = np.empty((_B, _C, _D, _D, _D), np.float32)
    for r in range(_NCORE):
        b, q = divmod(r, 4)
        out[b, q * _CL:(q + 1) * _CL] = np.asarray(
            res.results[r]["out"], dtype=np.float32
        )

    if res.exec_time_ns is not None:
        print(f"HW exec time: {res.exec_time_ns} ns")
    return out
